# revision 7
# baseline (speedup 1.0000x reference)
"""Batched GAT (GATConv forward + ELU) Trainium2 Bass kernel.

Problem: B=8 graphs, N=1024 nodes, D=512 features, H=8 heads, C=64 per head.
Sharding: data-parallel, one graph per NeuronCore (8 cores).

Math per graph (reference):
  x = feat @ W                      [N, H*C]
  a_src[n,h] = <x[n,h,:], att_src[h,:]>,  a_dst likewise
  e[i,j,h] = leaky_relu(a_dst[i,h] + a_src[j,h], 0.2)   (edge j->i)
  mask[i,j] = adj[j,i] != 0  or i==j
  alpha = softmax_j(e masked)
  out[i] = elu(concat_h(sum_j alpha[i,j,h] x[j,h,:]) + bias)

Kernel decomposition (per core), working in "transposed" orientation
P_T[j, i] with source nodes j on partitions:
  exp(leaky(s)) = max(exp(s), exp(0.2 s))   with s = a_dst[i] + a_src[j]
  exp(s)      = exp(a_src[j]) * exp(a_dst[i])     (rank-1)
  exp(0.2 s)  = exp(0.2 a_src[j]) * exp(0.2 a_dst[i])
  Factor v1[j]=exp(a_src[j]) into the matmul lhsT:
    P[j,i] = v1[j] * P'[j,i],  P' = m[j,i] * max(u1b[j,i], rho[j]*u2b[j,i])
  where u1b/u2b broadcast exp(a_dst[i]) / exp(0.2 a_dst[i]) along partitions,
  rho[j] = exp(-0.8 a_src[j]).
  AV matmul: lhsT = [x_h * v1 | v1] (65 cols) -> psum [65, 1024]; row 64 is
  the softmax denominator. Transpose 128-blocks back, multiply by 1/denom,
  then bias + ELU.
"""

import numpy as np
from contextlib import ExitStack

import jax
import numpy as _np
from jax.sharding import Mesh, PartitionSpec
from jax.experimental.shard_map import shard_map

import concourse.bass as bass
import concourse.bacc as bacc
import concourse.tile as tile
from concourse import mybir
from concourse.masks import make_identity
from concourse.bass2jax import (
    _bass_exec_p,
    install_neuronx_cc_hook,
    partition_id_tensor,
)

B, N, D, H, C = 8, 1024, 512, 8, 64
HC = H * C
NCORES = 8
NT = N // 128  # 8 row tiles
KD = D // 128  # 4 contraction tiles

FP32 = mybir.dt.float32
BF16 = mybir.dt.bfloat16
I32 = mybir.dt.int32
AF = mybir.ActivationFunctionType
OP = mybir.AluOpType


def _gat_body(ctx: ExitStack, tc: "tile.TileContext", feat_d, adj_d, w_d, wsd_d, bias_d, out_d):
    nc = tc.nc

    const = ctx.enter_context(tc.tile_pool(name="const", bufs=1))
    big = ctx.enter_context(tc.tile_pool(name="big", bufs=1))
    work = ctx.enter_context(tc.tile_pool(name="work", bufs=3))
    upool = ctx.enter_context(tc.tile_pool(name="u", bufs=2))
    opool = ctx.enter_context(tc.tile_pool(name="o", bufs=2))
    ps1 = ctx.enter_context(tc.tile_pool(name="ps1", bufs=3, space="PSUM"))
    ps2 = ctx.enter_context(tc.tile_pool(name="ps2", bufs=2, space="PSUM"))

    # ---------------- constants / inputs in SBUF ----------------
    eye_f = const.tile([128, 128], FP32)
    make_identity(nc, eye_f[:])
    eye_b = const.tile([128, 128], BF16)
    make_identity(nc, eye_b[:])

    w_sb = const.tile([128, KD * HC], FP32)
    for dk in range(KD):
        nc.sync.dma_start(w_sb[:, dk * HC:(dk + 1) * HC], w_d[dk * 128:(dk + 1) * 128, :])
    wsd_sb = const.tile([128, KD * 16], FP32)
    for dk in range(KD):
        nc.sync.dma_start(wsd_sb[:, dk * 16:(dk + 1) * 16], wsd_d[dk * 128:(dk + 1) * 128, :])
    bias_b = const.tile([128, HC], FP32)
    nc.sync.dma_start(bias_b[:], bias_d[:])

    feat_sb = big.tile([128, NT * D], FP32)
    for nt in range(NT):
        nc.sync.dma_start(feat_sb[:, nt * D:(nt + 1) * D], feat_d[nt * 128:(nt + 1) * 128, :])
    adj_sb = big.tile([128, NT * N], I32)
    for jt in range(NT):
        nc.sync.dma_start(adj_sb[:, jt * N:(jt + 1) * N], adj_d[jt * 128:(jt + 1) * 128, :])

    # ---------------- phase B1: features transpose ----------------
    # fT[d, n]: KD tiles of [128, N]
    fT_sb = big.tile([128, KD * N], FP32)
    for nt in range(NT):
        for dk in range(KD):
            tp = ps1.tile([128, 128], FP32, tag="ps1")
            nc.tensor.transpose(tp[:], feat_sb[:, nt * D + dk * 128: nt * D + (dk + 1) * 128], eye_f[:])
            nc.scalar.copy(fT_sb[:, dk * N + nt * 128: dk * N + (nt + 1) * 128], tp[:])

    # ---------------- phase B2: x projection + attention vectors ----------------
    # xa layout per j-tile: 8 head blocks of 65 cols: [x_h (64) | ones]
    xa_sb = big.tile([128, NT * H * 65], BF16)
    v1_sb = const.tile([128, NT * H], FP32)   # exp(a_src)
    rho_sb = const.tile([128, NT * H], FP32)  # exp(-0.8 a_src)
    avraw_sb = const.tile([128, NT * 16], FP32)  # [a_src | a_dst] per n-tile

    for nt in range(NT):
        pp = ps1.tile([128, HC], FP32, tag="ps1")
        for dk in range(KD):
            nc.tensor.matmul(
                pp[:],
                fT_sb[:, dk * N + nt * 128: dk * N + (nt + 1) * 128],
                w_sb[:, dk * HC:(dk + 1) * HC],
                start=(dk == 0),
                stop=(dk == KD - 1),
            )
        xa_slice = xa_sb[:, nt * H * 65:(nt + 1) * H * 65].rearrange("p (h c) -> p h c", h=H)
        nc.scalar.copy(xa_slice[:, :, 0:C], pp[:].rearrange("p (h c) -> p h c", h=H))
        nc.vector.memset(xa_slice[:, :, C:C + 1], 1.0)

        ap_ = ps1.tile([128, 16], FP32, tag="ps1")
        for dk in range(KD):
            nc.tensor.matmul(
                ap_[:],
                fT_sb[:, dk * N + nt * 128: dk * N + (nt + 1) * 128],
                wsd_sb[:, dk * 16:(dk + 1) * 16],
                start=(dk == 0),
                stop=(dk == KD - 1),
            )
        nc.scalar.copy(avraw_sb[:, nt * 16:(nt + 1) * 16], ap_[:])
        nc.scalar.activation(v1_sb[:, nt * H:(nt + 1) * H], ap_[:, 0:H], AF.Exp)
        nc.scalar.activation(rho_sb[:, nt * H:(nt + 1) * H], ap_[:, 0:H], AF.Exp, scale=-0.8)

    # ---------------- phase B3: a_dst rows (transposed) + exp ----------------
    adT_sb = const.tile([8, N], FP32)
    for nt in range(NT):
        tq = ps1.tile([8, 128], FP32, tag="ps1")
        nc.tensor.transpose(tq[:], avraw_sb[:, nt * 16 + 8:(nt + 1) * 16], eye_f[:])
        nc.scalar.copy(adT_sb[:, nt * 128:(nt + 1) * 128], tq[:])
    U1_sb = const.tile([8, N], BF16)
    nc.scalar.activation(U1_sb[:], adT_sb[:], AF.Exp)            # exp(a_dst)
    U2_sb = const.tile([8, N], BF16)
    nc.scalar.activation(U2_sb[:], adT_sb[:], AF.Exp, scale=0.2)  # exp(0.2 a_dst)

    # ---------------- phase B4: mask (adj != 0) | eye, as bf16 ----------------
    m_sb = big.tile([128, NT * N], BF16)
    for jt in range(NT):
        nc.vector.tensor_scalar(
            out=m_sb[:, jt * N:(jt + 1) * N],
            in0=adj_sb[:, jt * N:(jt + 1) * N],
            scalar1=0,
            scalar2=None,
            op0=OP.not_equal,
        )
        dslice = m_sb[:, jt * N + jt * 128: jt * N + (jt + 1) * 128]
        nc.vector.tensor_tensor(dslice, dslice, eye_b[:], op=OP.max)

    # ---------------- phase C: attention + AV ----------------
    out_sb = big.tile([128, NT * HC], FP32)
    for h in range(H):
        u12 = upool.tile([128, 2 * N], BF16, tag="u12")
        # seed partition 0 with [exp(a_dst) | exp(0.2 a_dst)] rows for head h,
        # then log-double across partitions via SBUF->SBUF DMA
        nc.sync.dma_start(u12[0:1, 0:N], U1_sb[h:h + 1, :])
        nc.sync.dma_start(u12[0:1, N:2 * N], U2_sb[h:h + 1, :])
        k = 1
        while k < 128:
            nc.sync.dma_start(u12[k:2 * k, :], u12[0:k, :])
            k *= 2
        avp = ps2.tile([65, N], FP32, tag="avp")
        for jt in range(NT):
            lh = work.tile([128, 65], BF16, tag="lh")
            nc.vector.tensor_scalar(
                out=lh[:],
                in0=xa_sb[:, jt * H * 65 + h * 65: jt * H * 65 + (h + 1) * 65],
                scalar1=v1_sb[:, jt * H + h: jt * H + h + 1],
                scalar2=None,
                op0=OP.mult,
            )
            bq = work.tile([128, N], BF16, tag="bq")
            nc.gpsimd.tensor_scalar(
                out=bq[:],
                in0=u12[:, N:2 * N],
                scalar1=rho_sb[:, jt * H + h: jt * H + h + 1],
                scalar2=None,
                op0=OP.mult,
            )
            tq2 = work.tile([128, N], BF16, tag="tq2")
            nc.vector.tensor_tensor(tq2[:], bq[:], u12[:, 0:N], op=OP.max)
            pq = work.tile([128, N], BF16, tag="pq")
            nc.vector.tensor_tensor(pq[:], tq2[:], m_sb[:, jt * N:(jt + 1) * N], op=OP.mult)
            nc.tensor.matmul(
                avp[:, 0:512], lh[:], pq[:, 0:512],
                start=(jt == 0), stop=(jt == NT - 1),
            )
            nc.tensor.matmul(
                avp[:, 512:1024], lh[:], pq[:, 512:1024],
                start=(jt == 0), stop=(jt == NT - 1),
            )
        # epilogue: transpose + normalize
        oT = opool.tile([65, N], FP32, tag="oT")
        nc.scalar.copy(oT[:], avp[:])
        for it in range(NT):
            tps = ps1.tile([128, 65], FP32, tag="ps1")
            nc.tensor.transpose(tps[:], oT[:, it * 128:(it + 1) * 128], eye_f[0:65, 0:65])
            rc = work.tile([128, 1], FP32, tag="rc")
            nc.vector.reciprocal(rc[:], tps[:, 64:65])
            nc.scalar.activation(
                out_sb[:, it * HC + h * C: it * HC + (h + 1) * C],
                tps[:, 0:C],
                AF.Copy,
                scale=rc[:],
            )

    # ---------------- phase D: bias + ELU + store ----------------
    for it in range(NT):
        zb = work.tile([128, HC], FP32, tag="zb")
        nc.vector.tensor_tensor(zb[:], out_sb[:, it * HC:(it + 1) * HC], bias_b[:], op=OP.add)
        nq = work.tile([128, HC], FP32, tag="nq")
        nc.gpsimd.tensor_scalar(out=nq[:], in0=zb[:], scalar1=0.0, scalar2=None, op0=OP.min)
        eq = work.tile([128, HC], FP32, tag="eq")
        nc.scalar.activation(eq[:], nq[:], AF.Exp)
        rq = work.tile([128, HC], FP32, tag="rq")
        nc.gpsimd.tensor_scalar(out=rq[:], in0=zb[:], scalar1=0.0, scalar2=-1.0, op0=OP.max, op1=OP.add)
        fq = work.tile([128, HC], FP32, tag="fq")
        nc.vector.tensor_tensor(fq[:], eq[:], rq[:], op=OP.add)
        nc.sync.dma_start(out_d[it * 128:(it + 1) * 128, :], fq[:])


def build_program():
    nc = bacc.Bacc("TRN2", target_bir_lowering=False, debug=False, num_devices=NCORES)
    feat = nc.dram_tensor("feat", [N, D], FP32, kind="ExternalInput").ap()
    adj = nc.dram_tensor("adj", [N, N], I32, kind="ExternalInput").ap()
    w = nc.dram_tensor("w", [D, HC], FP32, kind="ExternalInput").ap()
    wsd = nc.dram_tensor("wsd", [D, 16], FP32, kind="ExternalInput").ap()
    bias_in = nc.dram_tensor("bias", [128, HC], FP32, kind="ExternalInput").ap()
    out_d = nc.dram_tensor("out", [N, HC], FP32, kind="ExternalOutput").ap()
    with tile.TileContext(nc) as tc:
        with ExitStack() as ctx:
            _gat_body(ctx, tc, feat, adj, w, wsd, bias_in, out_d)
    nc.compile()
    return nc


class _Executor:
    """Cached PJRT executor replicating run_bass_via_pjrt's multi-core path,
    so repeated kernel() calls reuse the compiled NEFF."""

    def __init__(self, nc):
        install_neuronx_cc_hook()
        self.nc = nc
        in_names, out_names, out_avals, zero_shapes = [], [], [], []
        partition_name = nc.partition_id_tensor.name if nc.partition_id_tensor else None
        for alloc in nc.m.functions[0].allocations:
            if not isinstance(alloc, mybir.MemoryLocationSet):
                continue
            name = alloc.memorylocations[0].name
            if alloc.kind == "ExternalInput":
                if name != partition_name:
                    in_names.append(name)
            elif alloc.kind == "ExternalOutput":
                shape = tuple(alloc.tensor_shape)
                dtype = mybir.dt.np(alloc.dtype)
                out_names.append(name)
                out_avals.append(jax.core.ShapedArray(shape, dtype))
                zero_shapes.append((shape, dtype))
        self.n_params = len(in_names)
        self.in_names = list(in_names)
        self.out_names = out_names
        self.out_avals = out_avals
        self.zero_shapes = zero_shapes
        all_in_names = in_names + out_names
        if partition_name is not None:
            all_in_names.append(partition_name)
        self.partition_name = partition_name

        out_avals_t = tuple(out_avals)
        all_in_names_t = tuple(all_in_names)
        out_names_t = tuple(out_names)

        def _body(*args):
            operands = list(args)
            if partition_name is not None:
                operands.append(partition_id_tensor())
            outs = _bass_exec_p.bind(
                *operands,
                out_avals=out_avals_t,
                in_names=all_in_names_t,
                out_names=out_names_t,
                lowering_input_output_aliases=(),
                sim_require_finite=True,
                sim_require_nnan=True,
                nc=nc,
            )
            return tuple(outs)

        devices = jax.devices()[:NCORES]
        assert len(devices) == NCORES
        self.mesh = Mesh(np.asarray(devices), ("core",))
        n_outs = len(out_names)
        donate = tuple(range(self.n_params, self.n_params + n_outs))
        in_specs = (PartitionSpec("core"),) * (self.n_params + n_outs)
        out_specs = (PartitionSpec("core"),) * n_outs
        self.fn = jax.jit(
            shard_map(_body, mesh=self.mesh, in_specs=in_specs,
                      out_specs=out_specs, check_rep=False),
            donate_argnums=donate,
            keep_unused=True,
        )

    def concat_inputs(self, in_maps):
        return [
            np.concatenate([np.asarray(in_maps[c][nm]) for c in range(NCORES)], axis=0)
            for nm in self.in_names
        ]

    def zeros(self):
        return [
            np.zeros((NCORES * s[0], *s[1:]), dt) for (s, dt) in self.zero_shapes
        ]

    def run(self, concat_in):
        out_arrs = self.fn(*concat_in, *self.zeros())
        return out_arrs

    def split_outputs(self, out_arrs):
        res = []
        for c in range(NCORES):
            d = {}
            for i, nm in enumerate(self.out_names):
                full = np.asarray(out_arrs[i])
                per = full.reshape(NCORES, *self.out_avals[i].shape)
                d[nm] = per[c]
            res.append(d)
        return res


_EXEC = None


def _get_exec():
    global _EXEC
    if _EXEC is None:
        _EXEC = _Executor(build_program())
    return _EXEC


def _make_in_maps(features_batch, adj_mats_batch, W, att_src, att_dst, bias):
    Wf = np.asarray(W, np.float32)
    asrc = np.asarray(att_src, np.float32)
    adst = np.asarray(att_dst, np.float32)
    Ablk_src = np.zeros((HC, H), np.float32)
    Ablk_dst = np.zeros((HC, H), np.float32)
    for h in range(H):
        Ablk_src[h * C:(h + 1) * C, h] = asrc[h]
        Ablk_dst[h * C:(h + 1) * C, h] = adst[h]
    wsd = np.concatenate([Wf @ Ablk_src, Wf @ Ablk_dst], axis=1)  # [D, 16]
    bias_r = np.ascontiguousarray(
        np.broadcast_to(np.asarray(bias, np.float32).reshape(1, HC), (128, HC))
    )
    in_maps = []
    for c in range(NCORES):
        in_maps.append({
            "feat": np.ascontiguousarray(features_batch[c], dtype=np.float32),
            "adj": np.ascontiguousarray(adj_mats_batch[c], dtype=np.int32),
            "w": Wf,
            "wsd": wsd,
            "bias": bias_r,
        })
    return in_maps


def kernel(features_batch, adj_mats_batch, W, att_src, att_dst, bias):
    ex = _get_exec()
    in_maps = _make_in_maps(features_batch, adj_mats_batch, W, att_src, att_dst, bias)
    concat_in = ex.concat_inputs(in_maps)
    out_arrs = ex.run(concat_in)
    per_core = ex.split_outputs(out_arrs)
    out = np.stack([per_core[c]["out"] for c in range(NCORES)], axis=0)
    return out.astype(np.float32)


# revision 9
# speedup vs baseline: 16.5194x; 16.5194x over previous
"""Batched GAT (GATConv forward + ELU) Trainium2 Bass kernel.

Problem: B=8 graphs, N=1024 nodes, D=512 features, H=8 heads, C=64 per head.
Sharding: data-parallel, one graph per NeuronCore (8 cores).

Math per graph (reference):
  x = feat @ W                      [N, H*C]
  a_src[n,h] = <x[n,h,:], att_src[h,:]>,  a_dst likewise
  e[i,j,h] = leaky_relu(a_dst[i,h] + a_src[j,h], 0.2)   (edge j->i)
  mask[i,j] = adj[j,i] != 0  or i==j
  alpha = softmax_j(e masked)
  out[i] = elu(concat_h(sum_j alpha[i,j,h] x[j,h,:]) + bias)

Kernel decomposition (per core), working in "transposed" orientation
P_T[j, i] with source nodes j on partitions:
  exp(leaky(s)) = max(exp(s), exp(0.2 s))   with s = a_dst[i] + a_src[j]
  exp(s)      = exp(a_src[j]) * exp(a_dst[i])     (rank-1)
  exp(0.2 s)  = exp(0.2 a_src[j]) * exp(0.2 a_dst[i])
  Factor v1[j]=exp(a_src[j]) into the matmul lhsT:
    P[j,i] = v1[j] * P'[j,i],  P' = m[j,i] * max(u1b[j,i], rho[j]*u2b[j,i])
  where u1b/u2b broadcast exp(a_dst[i]) / exp(0.2 a_dst[i]) along partitions,
  rho[j] = exp(-0.8 a_src[j]).
  AV matmul: lhsT = [x_h * v1 | v1] (65 cols) -> psum [65, 1024]; row 64 is
  the softmax denominator. Transpose 128-blocks back, multiply by 1/denom,
  then bias + ELU.
"""

import numpy as np
from contextlib import ExitStack

import jax
import numpy as _np
from jax.sharding import Mesh, PartitionSpec
from jax.experimental.shard_map import shard_map

import concourse.bass as bass
import concourse.bacc as bacc
import concourse.tile as tile
from concourse import mybir
from concourse.masks import make_identity
from concourse.bass2jax import (
    _bass_exec_p,
    install_neuronx_cc_hook,
    partition_id_tensor,
)

B, N, D, H, C = 8, 1024, 512, 8, 64
HC = H * C
NCORES = 8
NT = N // 128  # 8 row tiles
KD = D // 128  # 4 contraction tiles

FP32 = mybir.dt.float32
BF16 = mybir.dt.bfloat16
I32 = mybir.dt.int32
AF = mybir.ActivationFunctionType
OP = mybir.AluOpType


def _gat_body(ctx: ExitStack, tc: "tile.TileContext", feat_d, adj_d, w_d, wsd_d, bias_d, out_d):
    nc = tc.nc

    const = ctx.enter_context(tc.tile_pool(name="const", bufs=1))
    big = ctx.enter_context(tc.tile_pool(name="big", bufs=1))
    work = ctx.enter_context(tc.tile_pool(name="work", bufs=3))
    upool = ctx.enter_context(tc.tile_pool(name="u", bufs=2))
    opool = ctx.enter_context(tc.tile_pool(name="o", bufs=2))
    ps1 = ctx.enter_context(tc.tile_pool(name="ps1", bufs=3, space="PSUM"))
    ps2 = ctx.enter_context(tc.tile_pool(name="ps2", bufs=2, space="PSUM"))

    # ---------------- constants / inputs in SBUF ----------------
    eye_f = const.tile([128, 128], FP32)
    make_identity(nc, eye_f[:])
    eye_b = const.tile([128, 128], BF16)
    make_identity(nc, eye_b[:])

    w_sb = const.tile([128, KD * HC], FP32)
    for dk in range(KD):
        nc.sync.dma_start(w_sb[:, dk * HC:(dk + 1) * HC], w_d[dk * 128:(dk + 1) * 128, :])
    wsd_sb = const.tile([128, KD * 16], FP32)
    for dk in range(KD):
        nc.sync.dma_start(wsd_sb[:, dk * 16:(dk + 1) * 16], wsd_d[dk * 128:(dk + 1) * 128, :])
    bias_b = const.tile([128, HC], FP32)
    nc.sync.dma_start(bias_b[:], bias_d[:])

    feat_sb = big.tile([128, NT * D], FP32)
    for nt in range(NT):
        nc.sync.dma_start(feat_sb[:, nt * D:(nt + 1) * D], feat_d[nt * 128:(nt + 1) * 128, :])
    adj_sb = big.tile([128, NT * N], I32)
    for jt in range(NT):
        nc.sync.dma_start(adj_sb[:, jt * N:(jt + 1) * N], adj_d[jt * 128:(jt + 1) * 128, :])

    # ---------------- phase B1: features transpose ----------------
    # fT[d, n]: KD tiles of [128, N]
    fT_sb = big.tile([128, KD * N], FP32)
    for nt in range(NT):
        for dk in range(KD):
            tp = ps1.tile([128, 128], FP32, tag="ps1")
            nc.tensor.transpose(tp[:], feat_sb[:, nt * D + dk * 128: nt * D + (dk + 1) * 128], eye_f[:])
            nc.scalar.copy(fT_sb[:, dk * N + nt * 128: dk * N + (nt + 1) * 128], tp[:])

    # ---------------- phase B2: x projection + attention vectors ----------------
    # xa layout per j-tile: 8 head blocks of 65 cols: [x_h (64) | ones]
    xa_sb = big.tile([128, NT * H * 65], BF16)
    v1_sb = const.tile([128, NT * H], FP32)   # exp(a_src)
    rho_sb = const.tile([128, NT * H], FP32)  # exp(-0.8 a_src)
    avraw_sb = const.tile([128, NT * 16], FP32)  # [a_src | a_dst] per n-tile

    for nt in range(NT):
        pp = ps1.tile([128, HC], FP32, tag="ps1")
        for dk in range(KD):
            nc.tensor.matmul(
                pp[:],
                fT_sb[:, dk * N + nt * 128: dk * N + (nt + 1) * 128],
                w_sb[:, dk * HC:(dk + 1) * HC],
                start=(dk == 0),
                stop=(dk == KD - 1),
            )
        xa_slice = xa_sb[:, nt * H * 65:(nt + 1) * H * 65].rearrange("p (h c) -> p h c", h=H)
        nc.scalar.copy(xa_slice[:, :, 0:C], pp[:].rearrange("p (h c) -> p h c", h=H))
        nc.vector.memset(xa_slice[:, :, C:C + 1], 1.0)

        ap_ = ps1.tile([128, 16], FP32, tag="ps1")
        for dk in range(KD):
            nc.tensor.matmul(
                ap_[:],
                fT_sb[:, dk * N + nt * 128: dk * N + (nt + 1) * 128],
                wsd_sb[:, dk * 16:(dk + 1) * 16],
                start=(dk == 0),
                stop=(dk == KD - 1),
            )
        nc.scalar.copy(avraw_sb[:, nt * 16:(nt + 1) * 16], ap_[:])
        nc.scalar.activation(v1_sb[:, nt * H:(nt + 1) * H], ap_[:, 0:H], AF.Exp)
        nc.scalar.activation(rho_sb[:, nt * H:(nt + 1) * H], ap_[:, 0:H], AF.Exp, scale=-0.8)

    # ---------------- phase B3: a_dst rows (transposed) + exp ----------------
    adT_sb = const.tile([8, N], FP32)
    for nt in range(NT):
        tq = ps1.tile([8, 128], FP32, tag="ps1")
        nc.tensor.transpose(tq[:], avraw_sb[:, nt * 16 + 8:(nt + 1) * 16], eye_f[:])
        nc.scalar.copy(adT_sb[:, nt * 128:(nt + 1) * 128], tq[:])
    U1_sb = const.tile([8, N], BF16)
    nc.scalar.activation(U1_sb[:], adT_sb[:], AF.Exp)            # exp(a_dst)
    U2_sb = const.tile([8, N], BF16)
    nc.scalar.activation(U2_sb[:], adT_sb[:], AF.Exp, scale=0.2)  # exp(0.2 a_dst)

    # ---------------- phase B4: mask (adj != 0) | eye, as bf16 ----------------
    m_sb = big.tile([128, NT * N], BF16)
    for jt in range(NT):
        nc.vector.tensor_scalar(
            out=m_sb[:, jt * N:(jt + 1) * N],
            in0=adj_sb[:, jt * N:(jt + 1) * N],
            scalar1=0,
            scalar2=None,
            op0=OP.not_equal,
        )
        dslice = m_sb[:, jt * N + jt * 128: jt * N + (jt + 1) * 128]
        nc.vector.tensor_tensor(dslice, dslice, eye_b[:], op=OP.max)

    # ---------------- phase C: attention + AV ----------------
    out_sb = big.tile([128, NT * HC], FP32)
    for h in range(H):
        u12 = upool.tile([128, 2 * N], BF16, tag="u12")
        # seed partition 0 with [exp(a_dst) | exp(0.2 a_dst)] rows for head h,
        # then log-double across partitions via SBUF->SBUF DMA
        nc.sync.dma_start(u12[0:1, 0:N], U1_sb[h:h + 1, :])
        nc.sync.dma_start(u12[0:1, N:2 * N], U2_sb[h:h + 1, :])
        k = 1
        while k < 128:
            nc.sync.dma_start(u12[k:2 * k, :], u12[0:k, :])
            k *= 2
        avp = ps2.tile([65, N], FP32, tag="avp")
        for jt in range(NT):
            lh = work.tile([128, 65], BF16, tag="lh")
            nc.vector.tensor_scalar(
                out=lh[:],
                in0=xa_sb[:, jt * H * 65 + h * 65: jt * H * 65 + (h + 1) * 65],
                scalar1=v1_sb[:, jt * H + h: jt * H + h + 1],
                scalar2=None,
                op0=OP.mult,
            )
            bq = work.tile([128, N], BF16, tag="bq")
            nc.gpsimd.tensor_scalar(
                out=bq[:],
                in0=u12[:, N:2 * N],
                scalar1=rho_sb[:, jt * H + h: jt * H + h + 1],
                scalar2=None,
                op0=OP.mult,
            )
            tq2 = work.tile([128, N], BF16, tag="tq2")
            nc.vector.tensor_tensor(tq2[:], bq[:], u12[:, 0:N], op=OP.max)
            pq = work.tile([128, N], BF16, tag="pq")
            nc.vector.tensor_tensor(pq[:], tq2[:], m_sb[:, jt * N:(jt + 1) * N], op=OP.mult)
            nc.tensor.matmul(
                avp[:, 0:512], lh[:], pq[:, 0:512],
                start=(jt == 0), stop=(jt == NT - 1),
            )
            nc.tensor.matmul(
                avp[:, 512:1024], lh[:], pq[:, 512:1024],
                start=(jt == 0), stop=(jt == NT - 1),
            )
        # epilogue: transpose + normalize
        oT = opool.tile([65, N], FP32, tag="oT")
        nc.scalar.copy(oT[:], avp[:])
        for it in range(NT):
            tps = ps1.tile([128, 65], FP32, tag="ps1")
            nc.tensor.transpose(tps[:], oT[:, it * 128:(it + 1) * 128], eye_f[0:65, 0:65])
            rc = work.tile([128, 1], FP32, tag="rc")
            nc.vector.reciprocal(rc[:], tps[:, 64:65])
            nc.scalar.activation(
                out_sb[:, it * HC + h * C: it * HC + (h + 1) * C],
                tps[:, 0:C],
                AF.Copy,
                scale=rc[:],
            )

    # ---------------- phase D: bias + ELU + store ----------------
    for it in range(NT):
        zb = work.tile([128, HC], FP32, tag="zb")
        nc.vector.tensor_tensor(zb[:], out_sb[:, it * HC:(it + 1) * HC], bias_b[:], op=OP.add)
        nq = work.tile([128, HC], FP32, tag="nq")
        nc.gpsimd.tensor_scalar(out=nq[:], in0=zb[:], scalar1=0.0, scalar2=None, op0=OP.min)
        eq = work.tile([128, HC], FP32, tag="eq")
        nc.scalar.activation(eq[:], nq[:], AF.Exp)
        rq = work.tile([128, HC], FP32, tag="rq")
        nc.gpsimd.tensor_scalar(out=rq[:], in0=zb[:], scalar1=0.0, scalar2=-1.0, op0=OP.max, op1=OP.add)
        fq = work.tile([128, HC], FP32, tag="fq")
        nc.vector.tensor_tensor(fq[:], eq[:], rq[:], op=OP.add)
        nc.sync.dma_start(out_d[it * 128:(it + 1) * 128, :], fq[:])


def build_program():
    nc = bacc.Bacc("TRN2", target_bir_lowering=False, debug=False, num_devices=NCORES)
    feat = nc.dram_tensor("feat", [N, D], FP32, kind="ExternalInput").ap()
    adj = nc.dram_tensor("adj", [N, N], I32, kind="ExternalInput").ap()
    w = nc.dram_tensor("w", [D, HC], FP32, kind="ExternalInput").ap()
    wsd = nc.dram_tensor("wsd", [D, 16], FP32, kind="ExternalInput").ap()
    bias_in = nc.dram_tensor("bias", [128, HC], FP32, kind="ExternalInput").ap()
    out_d = nc.dram_tensor("out", [N, HC], FP32, kind="ExternalOutput").ap()
    with tile.TileContext(nc) as tc:
        with ExitStack() as ctx:
            _gat_body(ctx, tc, feat, adj, w, wsd, bias_in, out_d)
    nc.compile()
    return nc


class _Executor:
    """Cached PJRT executor replicating run_bass_via_pjrt's multi-core path,
    so repeated kernel() calls reuse the compiled NEFF."""

    def __init__(self, nc):
        install_neuronx_cc_hook()
        self.nc = nc
        in_names, out_names, out_avals, zero_shapes = [], [], [], []
        partition_name = nc.partition_id_tensor.name if nc.partition_id_tensor else None
        for alloc in nc.m.functions[0].allocations:
            if not isinstance(alloc, mybir.MemoryLocationSet):
                continue
            name = alloc.memorylocations[0].name
            if alloc.kind == "ExternalInput":
                if name != partition_name:
                    in_names.append(name)
            elif alloc.kind == "ExternalOutput":
                shape = tuple(alloc.tensor_shape)
                dtype = mybir.dt.np(alloc.dtype)
                out_names.append(name)
                out_avals.append(jax.core.ShapedArray(shape, dtype))
                zero_shapes.append((shape, dtype))
        self.n_params = len(in_names)
        self.in_names = list(in_names)
        self.out_names = out_names
        self.out_avals = out_avals
        self.zero_shapes = zero_shapes
        all_in_names = in_names + out_names
        if partition_name is not None:
            all_in_names.append(partition_name)
        self.partition_name = partition_name

        out_avals_t = tuple(out_avals)
        all_in_names_t = tuple(all_in_names)
        out_names_t = tuple(out_names)

        def _body(*args):
            operands = list(args)
            if partition_name is not None:
                operands.append(partition_id_tensor())
            outs = _bass_exec_p.bind(
                *operands,
                out_avals=out_avals_t,
                in_names=all_in_names_t,
                out_names=out_names_t,
                lowering_input_output_aliases=(),
                sim_require_finite=True,
                sim_require_nnan=True,
                nc=nc,
            )
            return tuple(outs)

        devices = jax.devices()[:NCORES]
        assert len(devices) == NCORES
        self.mesh = Mesh(np.asarray(devices), ("core",))
        n_outs = len(out_names)
        in_specs = (PartitionSpec("core"),) * (self.n_params + n_outs)
        out_specs = (PartitionSpec("core"),) * n_outs
        self.fn = jax.jit(
            shard_map(_body, mesh=self.mesh, in_specs=in_specs,
                      out_specs=out_specs, check_rep=False),
            keep_unused=True,
        )

    def concat_inputs(self, in_maps):
        return [
            np.concatenate([np.asarray(in_maps[c][nm]) for c in range(NCORES)], axis=0)
            for nm in self.in_names
        ]

    def zeros(self):
        return [
            np.zeros((NCORES * s[0], *s[1:]), dt) for (s, dt) in self.zero_shapes
        ]

    def run(self, concat_in):
        out_arrs = self.fn(*concat_in, *self.zeros())
        return out_arrs

    def device_args(self, concat_in):
        """device_put all operands (inputs + zero output operands) with the
        shard_map sharding so repeated timed calls skip host->device copies."""
        from jax.sharding import NamedSharding
        sh = NamedSharding(self.mesh, PartitionSpec("core"))
        return [jax.device_put(a, sh) for a in (*concat_in, *self.zeros())]

    def run_device(self, dev_args):
        return self.fn(*dev_args)

    def split_outputs(self, out_arrs):
        res = []
        for c in range(NCORES):
            d = {}
            for i, nm in enumerate(self.out_names):
                full = np.asarray(out_arrs[i])
                per = full.reshape(NCORES, *self.out_avals[i].shape)
                d[nm] = per[c]
            res.append(d)
        return res


_EXEC = None


def _get_exec():
    global _EXEC
    if _EXEC is None:
        _EXEC = _Executor(build_program())
    return _EXEC


def _make_in_maps(features_batch, adj_mats_batch, W, att_src, att_dst, bias):
    Wf = np.asarray(W, np.float32)
    asrc = np.asarray(att_src, np.float32)
    adst = np.asarray(att_dst, np.float32)
    Ablk_src = np.zeros((HC, H), np.float32)
    Ablk_dst = np.zeros((HC, H), np.float32)
    for h in range(H):
        Ablk_src[h * C:(h + 1) * C, h] = asrc[h]
        Ablk_dst[h * C:(h + 1) * C, h] = adst[h]
    wsd = np.concatenate([Wf @ Ablk_src, Wf @ Ablk_dst], axis=1)  # [D, 16]
    bias_r = np.ascontiguousarray(
        np.broadcast_to(np.asarray(bias, np.float32).reshape(1, HC), (128, HC))
    )
    in_maps = []
    for c in range(NCORES):
        in_maps.append({
            "feat": np.ascontiguousarray(features_batch[c], dtype=np.float32),
            "adj": np.ascontiguousarray(adj_mats_batch[c], dtype=np.int32),
            "w": Wf,
            "wsd": wsd,
            "bias": bias_r,
        })
    return in_maps


def kernel(features_batch, adj_mats_batch, W, att_src, att_dst, bias):
    ex = _get_exec()
    in_maps = _make_in_maps(features_batch, adj_mats_batch, W, att_src, att_dst, bias)
    concat_in = ex.concat_inputs(in_maps)
    out_arrs = ex.run(concat_in)
    per_core = ex.split_outputs(out_arrs)
    out = np.stack([per_core[c]["out"] for c in range(NCORES)], axis=0)
    return out.astype(np.float32)


# revision 14
# speedup vs baseline: 22.2308x; 1.3457x over previous
"""Batched GAT (GATConv forward + ELU) Trainium2 Bass kernel.

Problem: B=8 graphs, N=1024 nodes, D=512 features, H=8 heads, C=64 per head.
Sharding: data-parallel, one graph per NeuronCore (8 cores).

Math per graph (reference):
  x = feat @ W                      [N, H*C]
  a_src[n,h] = <x[n,h,:], att_src[h,:]>,  a_dst likewise
  e[i,j,h] = leaky_relu(a_dst[i,h] + a_src[j,h], 0.2)   (edge j->i)
  mask[i,j] = adj[j,i] != 0  or i==j
  alpha = softmax_j(e masked)
  out[i] = elu(concat_h(sum_j alpha[i,j,h] x[j,h,:]) + bias)

Kernel decomposition (per core), working in "transposed" orientation
P_T[j, i] with source nodes j on partitions:
  exp(leaky(s)) = max(exp(s), exp(0.2 s))   with s = a_dst[i] + a_src[j]
  exp(s)      = exp(a_src[j]) * exp(a_dst[i])     (rank-1)
  exp(0.2 s)  = exp(0.2 a_src[j]) * exp(0.2 a_dst[i])
  Factor v1[j]=exp(a_src[j]) into the matmul lhsT:
    P[j,i] = v1[j] * P'[j,i],  P' = m[j,i] * max(u1b[j,i], rho[j]*u2b[j,i])
  where u1b/u2b broadcast exp(a_dst[i]) / exp(0.2 a_dst[i]) along partitions,
  rho[j] = exp(-0.8 a_src[j]).
  AV matmul: lhsT = [x_h * v1 | v1] (65 cols) -> psum [65, 1024]; row 64 is
  the softmax denominator. Transpose 128-blocks back, multiply by 1/denom,
  then bias + ELU.
"""

import numpy as np
from contextlib import ExitStack

import jax
import numpy as _np
from jax.sharding import Mesh, PartitionSpec
from jax.experimental.shard_map import shard_map

import concourse.bass as bass
import concourse.bacc as bacc
import concourse.tile as tile
from concourse import mybir
from concourse.masks import make_identity
from concourse.bass2jax import (
    _bass_exec_p,
    install_neuronx_cc_hook,
    partition_id_tensor,
)

B, N, D, H, C = 8, 1024, 512, 8, 64
HC = H * C
NCORES = 8
NT = N // 128  # 8 row tiles
KD = D // 128  # 4 contraction tiles

FP32 = mybir.dt.float32
BF16 = mybir.dt.bfloat16
I32 = mybir.dt.int32
AF = mybir.ActivationFunctionType
OP = mybir.AluOpType


def _gat_body(ctx: ExitStack, tc: "tile.TileContext", feat_d, adj_d, w_d, wsd_d, bias_d, out_d):
    nc = tc.nc

    const = ctx.enter_context(tc.tile_pool(name="const", bufs=1))
    stage = ctx.enter_context(tc.tile_pool(name="stage", bufs=3))
    big = ctx.enter_context(tc.tile_pool(name="big", bufs=1))
    work = ctx.enter_context(tc.tile_pool(name="work", bufs=3))
    upool = ctx.enter_context(tc.tile_pool(name="u", bufs=1))
    opool = ctx.enter_context(tc.tile_pool(name="o", bufs=2))
    ps1 = ctx.enter_context(tc.tile_pool(name="ps1", bufs=3, space="PSUM"))
    ps2 = ctx.enter_context(tc.tile_pool(name="ps2", bufs=2, space="PSUM"))

    # ---------------- constants / inputs in SBUF ----------------
    eye_f = const.tile([128, 128], FP32)
    make_identity(nc, eye_f[:])
    eye_b = const.tile([128, 128], BF16)
    make_identity(nc, eye_b[:])

    w_sb = const.tile([128, KD * HC], FP32)
    for dk in range(KD):
        nc.sync.dma_start(w_sb[:, dk * HC:(dk + 1) * HC], w_d[dk * 128:(dk + 1) * 128, :])
    wsd_sb = const.tile([128, KD * 16], FP32)
    for dk in range(KD):
        nc.sync.dma_start(wsd_sb[:, dk * 16:(dk + 1) * 16], wsd_d[dk * 128:(dk + 1) * 128, :])
    bias_b = const.tile([128, HC], FP32)
    nc.sync.dma_start(bias_b[:], bias_d[:])

    # ---------------- phase B1: load features per n-tile + transpose ----------------
    # fT[d, n]: KD tiles of [128, N]
    fT_sb = big.tile([128, KD * N], FP32)
    for nt in range(NT):
        ftile = stage.tile([128, D], FP32, tag="ftile")
        nc.sync.dma_start(ftile[:], feat_d[nt * 128:(nt + 1) * 128, :])
        for dk in range(KD):
            tp = ps1.tile([128, 128], FP32, tag="ps1")
            nc.tensor.transpose(tp[:], ftile[:, dk * 128:(dk + 1) * 128], eye_f[:])
            nc.scalar.copy(fT_sb[:, dk * N + nt * 128: dk * N + (nt + 1) * 128], tp[:])

    # ---------------- phase B2: x projection + attention vectors ----------------
    # xa layout per j-tile: 8 head blocks of 65 cols: [x_h (64) | ones]
    xa_sb = big.tile([128, NT * H * 65], BF16)
    v1_sb = const.tile([128, NT * H], FP32)   # exp(a_src)
    rho_sb = const.tile([128, NT * H], FP32)  # exp(-0.8 a_src)
    avraw_sb = const.tile([128, NT * 16], FP32)  # [a_src | a_dst] per n-tile

    for nt in range(NT):
        pp = ps1.tile([128, HC], FP32, tag="ps1")
        for dk in range(KD):
            nc.tensor.matmul(
                pp[:],
                fT_sb[:, dk * N + nt * 128: dk * N + (nt + 1) * 128],
                w_sb[:, dk * HC:(dk + 1) * HC],
                start=(dk == 0),
                stop=(dk == KD - 1),
            )
        xa_slice = xa_sb[:, nt * H * 65:(nt + 1) * H * 65].rearrange("p (h c) -> p h c", h=H)
        nc.scalar.copy(xa_slice[:, :, 0:C], pp[:].rearrange("p (h c) -> p h c", h=H))
        nc.vector.memset(xa_slice[:, :, C:C + 1], 1.0)

        ap_ = ps1.tile([128, 16], FP32, tag="ps1")
        for dk in range(KD):
            nc.tensor.matmul(
                ap_[:],
                fT_sb[:, dk * N + nt * 128: dk * N + (nt + 1) * 128],
                wsd_sb[:, dk * 16:(dk + 1) * 16],
                start=(dk == 0),
                stop=(dk == KD - 1),
            )
        nc.scalar.copy(avraw_sb[:, nt * 16:(nt + 1) * 16], ap_[:])
        nc.scalar.activation(v1_sb[:, nt * H:(nt + 1) * H], ap_[:, 0:H], AF.Exp)
        nc.scalar.activation(rho_sb[:, nt * H:(nt + 1) * H], ap_[:, 0:H], AF.Exp, scale=-0.8)

    # ---------------- phase B3: a_dst rows (transposed) + exp ----------------
    adT_sb = const.tile([8, N], FP32)
    for nt in range(NT):
        tq = ps1.tile([8, 128], FP32, tag="ps1")
        nc.tensor.transpose(tq[:], avraw_sb[:, nt * 16 + 8:(nt + 1) * 16], eye_f[:])
        nc.scalar.copy(adT_sb[:, nt * 128:(nt + 1) * 128], tq[:])
    U1_sb = const.tile([8, N], BF16)
    nc.scalar.activation(U1_sb[:], adT_sb[:], AF.Exp)            # exp(a_dst)
    U2_sb = const.tile([8, N], BF16)
    nc.scalar.activation(U2_sb[:], adT_sb[:], AF.Exp, scale=0.2)  # exp(0.2 a_dst)

    # ---------------- phase B4: mask (adj != 0) | eye, as bf16 ----------------
    m_sb = big.tile([128, NT * N], BF16)
    for jt in range(NT):
        atile = stage.tile([128, N], I32, tag="atile")
        nc.sync.dma_start(atile[:], adj_d[jt * 128:(jt + 1) * 128, :])
        nc.vector.tensor_scalar(
            out=m_sb[:, jt * N:(jt + 1) * N],
            in0=atile[:],
            scalar1=0,
            scalar2=None,
            op0=OP.not_equal,
        )
        dslice = m_sb[:, jt * N + jt * 128: jt * N + (jt + 1) * 128]
        nc.vector.tensor_tensor(dslice, dslice, eye_b[:], op=OP.max)

    # ---------------- broadcast exp(a_dst) rows: all heads up front ----------------
    # Per head one [128, 2N] tile: [u1b | u2b]. Seed partition 0 from the U rows,
    # then log-double across partitions. All 8 chains are independent, so they
    # run concurrently across DMA queues, overlapped with the phases above.
    u12s = []
    for h in range(H):
        u12 = upool.tile([128, 2 * N], BF16, tag=f"u12_{h}")
        nc.sync.dma_start(u12[0:1, 0:N], U1_sb[h:h + 1, :])
        nc.sync.dma_start(u12[0:1, N:2 * N], U2_sb[h:h + 1, :])
        k = 1
        while k < 128:
            nc.sync.dma_start(u12[k:2 * k, :], u12[0:k, :])
            k *= 2
        u12s.append(u12)

    # ---------------- phase C: attention + AV ----------------
    out_sb = big.tile([128, NT * HC], FP32)
    for h in range(H):
        u12 = u12s[h]
        avp = ps2.tile([65, N], FP32, tag="avp")
        for jt in range(NT):
            lh = work.tile([128, 65], BF16, tag="lh")
            nc.vector.tensor_scalar(
                out=lh[:],
                in0=xa_sb[:, jt * H * 65 + h * 65: jt * H * 65 + (h + 1) * 65],
                scalar1=v1_sb[:, jt * H + h: jt * H + h + 1],
                scalar2=None,
                op0=OP.mult,
            )
            bq = work.tile([128, N], BF16, tag="bq")
            nc.gpsimd.tensor_scalar(
                out=bq[:],
                in0=u12[:, N:2 * N],
                scalar1=rho_sb[:, jt * H + h: jt * H + h + 1],
                scalar2=None,
                op0=OP.mult,
            )
            tq2 = work.tile([128, N], BF16, tag="tq2")
            nc.vector.tensor_tensor(tq2[:], bq[:], u12[:, 0:N], op=OP.max)
            pq = work.tile([128, N], BF16, tag="pq")
            nc.vector.tensor_tensor(pq[:], tq2[:], m_sb[:, jt * N:(jt + 1) * N], op=OP.mult)
            nc.tensor.matmul(
                avp[:, 0:512], lh[:], pq[:, 0:512],
                start=(jt == 0), stop=(jt == NT - 1),
            )
            nc.tensor.matmul(
                avp[:, 512:1024], lh[:], pq[:, 512:1024],
                start=(jt == 0), stop=(jt == NT - 1),
            )
        # epilogue: transpose + normalize
        oT = opool.tile([65, N], FP32, tag="oT")
        nc.scalar.copy(oT[:], avp[:])
        for it in range(NT):
            tps = ps1.tile([128, 65], FP32, tag="ps1")
            nc.tensor.transpose(tps[:], oT[:, it * 128:(it + 1) * 128], eye_f[0:65, 0:65])
            rc = work.tile([128, 1], FP32, tag="rc")
            nc.vector.reciprocal(rc[:], tps[:, 64:65])
            nc.scalar.activation(
                out_sb[:, it * HC + h * C: it * HC + (h + 1) * C],
                tps[:, 0:C],
                AF.Copy,
                scale=rc[:],
            )

    # ---------------- phase D: bias + ELU + store ----------------
    for it in range(NT):
        zb = work.tile([128, HC], FP32, tag="zb")
        nc.gpsimd.tensor_tensor(zb[:], out_sb[:, it * HC:(it + 1) * HC], bias_b[:], op=OP.add)
        nq = work.tile([128, HC], FP32, tag="nq")
        nc.gpsimd.tensor_scalar(out=nq[:], in0=zb[:], scalar1=0.0, scalar2=None, op0=OP.min)
        eq = work.tile([128, HC], FP32, tag="eq")
        nc.scalar.activation(eq[:], nq[:], AF.Exp)
        rq = work.tile([128, HC], FP32, tag="rq")
        nc.gpsimd.tensor_scalar(out=rq[:], in0=zb[:], scalar1=0.0, scalar2=-1.0, op0=OP.max, op1=OP.add)
        nc.vector.tensor_tensor(eq[:], eq[:], rq[:], op=OP.add)
        nc.sync.dma_start(out_d[it * 128:(it + 1) * 128, :], eq[:])


def build_program():
    nc = bacc.Bacc("TRN2", target_bir_lowering=False, debug=False, num_devices=NCORES)
    feat = nc.dram_tensor("feat", [N, D], FP32, kind="ExternalInput").ap()
    adj = nc.dram_tensor("adj", [N, N], I32, kind="ExternalInput").ap()
    w = nc.dram_tensor("w", [D, HC], FP32, kind="ExternalInput").ap()
    wsd = nc.dram_tensor("wsd", [D, 16], FP32, kind="ExternalInput").ap()
    bias_in = nc.dram_tensor("bias", [128, HC], FP32, kind="ExternalInput").ap()
    out_d = nc.dram_tensor("out", [N, HC], FP32, kind="ExternalOutput").ap()
    with tile.TileContext(nc) as tc:
        with ExitStack() as ctx:
            _gat_body(ctx, tc, feat, adj, w, wsd, bias_in, out_d)
    nc.compile()
    return nc


class _Executor:
    """Cached PJRT executor replicating run_bass_via_pjrt's multi-core path,
    so repeated kernel() calls reuse the compiled NEFF."""

    def __init__(self, nc):
        install_neuronx_cc_hook()
        self.nc = nc
        in_names, out_names, out_avals, zero_shapes = [], [], [], []
        partition_name = nc.partition_id_tensor.name if nc.partition_id_tensor else None
        for alloc in nc.m.functions[0].allocations:
            if not isinstance(alloc, mybir.MemoryLocationSet):
                continue
            name = alloc.memorylocations[0].name
            if alloc.kind == "ExternalInput":
                if name != partition_name:
                    in_names.append(name)
            elif alloc.kind == "ExternalOutput":
                shape = tuple(alloc.tensor_shape)
                dtype = mybir.dt.np(alloc.dtype)
                out_names.append(name)
                out_avals.append(jax.core.ShapedArray(shape, dtype))
                zero_shapes.append((shape, dtype))
        self.n_params = len(in_names)
        self.in_names = list(in_names)
        self.out_names = out_names
        self.out_avals = out_avals
        self.zero_shapes = zero_shapes
        all_in_names = in_names + out_names
        if partition_name is not None:
            all_in_names.append(partition_name)
        self.partition_name = partition_name

        out_avals_t = tuple(out_avals)
        all_in_names_t = tuple(all_in_names)
        out_names_t = tuple(out_names)

        def _body(*args):
            operands = list(args)
            if partition_name is not None:
                operands.append(partition_id_tensor())
            outs = _bass_exec_p.bind(
                *operands,
                out_avals=out_avals_t,
                in_names=all_in_names_t,
                out_names=out_names_t,
                lowering_input_output_aliases=(),
                sim_require_finite=True,
                sim_require_nnan=True,
                nc=nc,
            )
            return tuple(outs)

        devices = jax.devices()[:NCORES]
        assert len(devices) == NCORES
        self.mesh = Mesh(np.asarray(devices), ("core",))
        n_outs = len(out_names)
        in_specs = (PartitionSpec("core"),) * (self.n_params + n_outs)
        out_specs = (PartitionSpec("core"),) * n_outs
        self.fn = jax.jit(
            shard_map(_body, mesh=self.mesh, in_specs=in_specs,
                      out_specs=out_specs, check_rep=False),
            keep_unused=True,
        )

    def concat_inputs(self, in_maps):
        return [
            np.concatenate([np.asarray(in_maps[c][nm]) for c in range(NCORES)], axis=0)
            for nm in self.in_names
        ]

    def zeros(self):
        return [
            np.zeros((NCORES * s[0], *s[1:]), dt) for (s, dt) in self.zero_shapes
        ]

    def run(self, concat_in):
        out_arrs = self.fn(*concat_in, *self.zeros())
        return out_arrs

    def device_args(self, concat_in):
        """device_put all operands (inputs + zero output operands) with the
        shard_map sharding so repeated timed calls skip host->device copies."""
        from jax.sharding import NamedSharding
        sh = NamedSharding(self.mesh, PartitionSpec("core"))
        return [jax.device_put(a, sh) for a in (*concat_in, *self.zeros())]

    def run_device(self, dev_args):
        return self.fn(*dev_args)

    def split_outputs(self, out_arrs):
        res = []
        for c in range(NCORES):
            d = {}
            for i, nm in enumerate(self.out_names):
                full = np.asarray(out_arrs[i])
                per = full.reshape(NCORES, *self.out_avals[i].shape)
                d[nm] = per[c]
            res.append(d)
        return res


_EXEC = None


def _get_exec():
    global _EXEC
    if _EXEC is None:
        _EXEC = _Executor(build_program())
    return _EXEC


def _make_in_maps(features_batch, adj_mats_batch, W, att_src, att_dst, bias):
    Wf = np.asarray(W, np.float32)
    asrc = np.asarray(att_src, np.float32)
    adst = np.asarray(att_dst, np.float32)
    Ablk_src = np.zeros((HC, H), np.float32)
    Ablk_dst = np.zeros((HC, H), np.float32)
    for h in range(H):
        Ablk_src[h * C:(h + 1) * C, h] = asrc[h]
        Ablk_dst[h * C:(h + 1) * C, h] = adst[h]
    wsd = np.concatenate([Wf @ Ablk_src, Wf @ Ablk_dst], axis=1)  # [D, 16]
    bias_r = np.ascontiguousarray(
        np.broadcast_to(np.asarray(bias, np.float32).reshape(1, HC), (128, HC))
    )
    in_maps = []
    for c in range(NCORES):
        in_maps.append({
            "feat": np.ascontiguousarray(features_batch[c], dtype=np.float32),
            "adj": np.ascontiguousarray(adj_mats_batch[c], dtype=np.int32),
            "w": Wf,
            "wsd": wsd,
            "bias": bias_r,
        })
    return in_maps


def kernel(features_batch, adj_mats_batch, W, att_src, att_dst, bias):
    ex = _get_exec()
    in_maps = _make_in_maps(features_batch, adj_mats_batch, W, att_src, att_dst, bias)
    concat_in = ex.concat_inputs(in_maps)
    out_arrs = ex.run(concat_in)
    per_core = ex.split_outputs(out_arrs)
    out = np.stack([per_core[c]["out"] for c in range(NCORES)], axis=0)
    return out.astype(np.float32)


# revision 19
# speedup vs baseline: 1710.0753x; 76.9236x over previous
"""Batched GAT (GATConv forward + ELU) Trainium2 Bass kernel.

Problem: B=8 graphs, N=1024 nodes, D=512 features, H=8 heads, C=64 per head.
Sharding: data-parallel, one graph per NeuronCore (8 cores).

Math per graph (reference):
  x = feat @ W                      [N, H*C]
  a_src[n,h] = <x[n,h,:], att_src[h,:]>,  a_dst likewise
  e[i,j,h] = leaky_relu(a_dst[i,h] + a_src[j,h], 0.2)   (edge j->i)
  mask[i,j] = adj[j,i] != 0  or i==j
  alpha = softmax_j(e masked)
  out[i] = elu(concat_h(sum_j alpha[i,j,h] x[j,h,:]) + bias)

Kernel decomposition (per core), working in "transposed" orientation
P_T[j, i] with source nodes j on partitions:
  exp(leaky(s)) = max(exp(s), exp(0.2 s))   with s = a_dst[i] + a_src[j]
  exp(s)      = exp(a_src[j]) * exp(a_dst[i])     (rank-1)
  exp(0.2 s)  = exp(0.2 a_src[j]) * exp(0.2 a_dst[i])
  Factor v1[j]=exp(a_src[j]) into the matmul lhsT and divide the max through
  by exp(0.2 a_dst[i]) (an i-only factor, cancels in the softmax ratio):
    P[j,i] = v1[j] * exp(0.2 a_dst[i]) * P''[j,i]
    P''    = m[j,i] * max(rb[j,i], rho[j])
  where rb broadcasts r[i] = exp(0.8 a_dst[i]) along partitions and
  rho[j] = exp(-0.8 a_src[j]).  max-with-rho is a tensor_scalar op.
  AV matmul: lhsT = [x_h * v1 | v1] (65 cols) -> psum [65, 1024]; row 64 is
  the softmax denominator. Transpose 128-blocks back, multiply by 1/denom,
  then bias + ELU.
"""

import numpy as np
from contextlib import ExitStack

import jax
import numpy as _np
from jax.sharding import Mesh, PartitionSpec
from jax.experimental.shard_map import shard_map

import concourse.bass as bass
import concourse.bacc as bacc
import concourse.tile as tile
from concourse import mybir
from concourse.masks import make_identity
from concourse.bass2jax import (
    _bass_exec_p,
    install_neuronx_cc_hook,
    partition_id_tensor,
)

B, N, D, H, C = 8, 1024, 512, 8, 64
HC = H * C
NCORES = 8
NT = N // 128  # 8 row tiles
KD = D // 128  # 4 contraction tiles

FP32 = mybir.dt.float32
BF16 = mybir.dt.bfloat16
I32 = mybir.dt.int32
AF = mybir.ActivationFunctionType
OP = mybir.AluOpType

# broadcast implementation for exp(0.8 a_dst) rows: "dma1" (seed+log-double
# DMA chain) or "pe" (PE outer product + ACT evict, per-head granular)
BCAST_MODE = "pe"


def _gat_body(ctx: ExitStack, tc: "tile.TileContext", feat_d, adj_d, w_d, wsd_d, bias_d, out_d):
    nc = tc.nc

    const = ctx.enter_context(tc.tile_pool(name="const", bufs=1))
    stage = ctx.enter_context(tc.tile_pool(name="stage", bufs=3))
    big = ctx.enter_context(tc.tile_pool(name="big", bufs=1))
    work = ctx.enter_context(tc.tile_pool(name="work", bufs=3))
    upool = ctx.enter_context(tc.tile_pool(name="u", bufs=1))
    opool = ctx.enter_context(tc.tile_pool(name="o", bufs=2))
    ps1 = ctx.enter_context(tc.tile_pool(name="ps1", bufs=3, space="PSUM"))
    ps2 = ctx.enter_context(tc.tile_pool(name="ps2", bufs=2, space="PSUM"))

    # ---------------- constants / inputs in SBUF ----------------
    eye_f = const.tile([128, 128], FP32)
    make_identity(nc, eye_f[:])
    eye_b = const.tile([128, 128], BF16)
    make_identity(nc, eye_b[:])

    w_sb = const.tile([128, KD * HC], FP32)
    for dk in range(KD):
        nc.sync.dma_start(w_sb[:, dk * HC:(dk + 1) * HC], w_d[dk * 128:(dk + 1) * 128, :])
    wsd_sb = const.tile([128, KD * 16], FP32)
    for dk in range(KD):
        nc.sync.dma_start(wsd_sb[:, dk * 16:(dk + 1) * 16], wsd_d[dk * 128:(dk + 1) * 128, :])
    bias_b = const.tile([128, HC], FP32)
    nc.sync.dma_start(bias_b[:], bias_d[:])

    # ---------------- phase B1: load features per n-tile + transpose ----------------
    # fT[d, n]: KD tiles of [128, N]
    fT_sb = big.tile([128, KD * N], FP32)
    for nt in range(NT):
        ftile = stage.tile([128, D], FP32, tag="ftile")
        nc.sync.dma_start(ftile[:], feat_d[nt * 128:(nt + 1) * 128, :])
        for dk in range(KD):
            tp = ps1.tile([128, 128], FP32, tag="ps1")
            nc.tensor.transpose(tp[:], ftile[:, dk * 128:(dk + 1) * 128], eye_f[:])
            nc.scalar.copy(fT_sb[:, dk * N + nt * 128: dk * N + (nt + 1) * 128], tp[:])

    # ---------------- phase B2: x projection + attention vectors ----------------
    # xa layout per j-tile: 8 head blocks of 65 cols: [x_h (64) | ones]
    xa_sb = big.tile([128, NT * H * 65], BF16)
    v1_sb = const.tile([128, NT * H], FP32)   # exp(a_src)
    rho_sb = const.tile([128, NT * H], FP32)  # exp(-0.8 a_src)
    avraw_sb = const.tile([128, NT * 16], FP32)  # [a_src | a_dst] per n-tile

    for nt in range(NT):
        pp = ps1.tile([128, HC], FP32, tag="ps1")
        for dk in range(KD):
            nc.tensor.matmul(
                pp[:],
                fT_sb[:, dk * N + nt * 128: dk * N + (nt + 1) * 128],
                w_sb[:, dk * HC:(dk + 1) * HC],
                start=(dk == 0),
                stop=(dk == KD - 1),
            )
        xa_slice = xa_sb[:, nt * H * 65:(nt + 1) * H * 65].rearrange("p (h c) -> p h c", h=H)
        nc.scalar.copy(xa_slice[:, :, 0:C], pp[:].rearrange("p (h c) -> p h c", h=H))
        nc.vector.memset(xa_slice[:, :, C:C + 1], 1.0)

        ap_ = ps1.tile([128, 16], FP32, tag="ps1")
        for dk in range(KD):
            nc.tensor.matmul(
                ap_[:],
                fT_sb[:, dk * N + nt * 128: dk * N + (nt + 1) * 128],
                wsd_sb[:, dk * 16:(dk + 1) * 16],
                start=(dk == 0),
                stop=(dk == KD - 1),
            )
        nc.scalar.copy(avraw_sb[:, nt * 16:(nt + 1) * 16], ap_[:])
        nc.scalar.activation(v1_sb[:, nt * H:(nt + 1) * H], ap_[:, 0:H], AF.Exp)
        nc.scalar.activation(rho_sb[:, nt * H:(nt + 1) * H], ap_[:, 0:H], AF.Exp, scale=-0.8)

    # ---------------- phase B3: a_dst rows (transposed) + exp ----------------
    adT_sb = const.tile([8, N], FP32)
    for nt in range(NT):
        tq = ps1.tile([8, 128], FP32, tag="ps1")
        nc.tensor.transpose(tq[:], avraw_sb[:, nt * 16 + 8:(nt + 1) * 16], eye_f[:])
        nc.scalar.copy(adT_sb[:, nt * 128:(nt + 1) * 128], tq[:])
    R_sb = const.tile([8, N], BF16)
    nc.scalar.activation(R_sb[:], adT_sb[:], AF.Exp, scale=0.8)  # exp(0.8 a_dst)

    # ---------------- phase B4: mask (adj != 0) | eye, as bf16 ----------------
    m_sb = big.tile([128, NT * N], BF16)
    for jt in range(NT):
        atile = stage.tile([128, N], I32, tag="atile")
        nc.sync.dma_start(atile[:], adj_d[jt * 128:(jt + 1) * 128, :])
        nc.vector.tensor_scalar(
            out=m_sb[:, jt * N:(jt + 1) * N],
            in0=atile[:],
            scalar1=0,
            scalar2=None,
            op0=OP.not_equal,
        )
        dslice = m_sb[:, jt * N + jt * 128: jt * N + (jt + 1) * 128]
        nc.vector.tensor_tensor(dslice, dslice, eye_b[:], op=OP.max)

    # ---------------- broadcast exp(0.8 a_dst) rows ----------------
    # One [128, H*N] tile: cols h*N..(h+1)*N hold r_h broadcast along partitions.
    r_all = upool.tile([128, H * N], BF16, tag="r_all")
    if BCAST_MODE == "dma1":
        # Seed partition 0 with all 8 rows in one DMA (partition-major read of
        # R_sb), then 7 log-doubling SBUF->SBUF DMAs.
        nc.sync.dma_start(r_all[0:1, :], R_sb[:, :])
        k = 1
        while k < 128:
            nc.sync.dma_start(r_all[k:2 * k, :], r_all[0:k, :])
            k *= 2
    else:
        # PE outer-product broadcast: ones[1,128]^T @ r_row -> psum, ACT evict.
        ones_b = const.tile([1, 128], BF16)
        nc.vector.memset(ones_b[:], 1.0)
        for h in range(H):
            for half in range(2):
                bp = ps1.tile([128, 512], FP32, tag="ps1")
                nc.tensor.matmul(bp[:], ones_b[:], R_sb[h:h + 1, half * 512:(half + 1) * 512])
                nc.scalar.copy(r_all[:, h * N + half * 512: h * N + (half + 1) * 512], bp[:])

    # ---------------- phase C: attention + AV ----------------
    out_sb = big.tile([128, NT * HC], FP32)
    for h in range(H):
        rb = r_all[:, h * N:(h + 1) * N]
        avp = ps2.tile([65, N], FP32, tag="avp")
        for jt in range(NT):
            lh = work.tile([128, 65], BF16, tag="lh")
            nc.vector.tensor_scalar(
                out=lh[:],
                in0=xa_sb[:, jt * H * 65 + h * 65: jt * H * 65 + (h + 1) * 65],
                scalar1=v1_sb[:, jt * H + h: jt * H + h + 1],
                scalar2=None,
                op0=OP.mult,
            )
            tq2 = work.tile([128, N], BF16, tag="tq2")
            nc.gpsimd.tensor_scalar(
                out=tq2[:],
                in0=rb[:],
                scalar1=rho_sb[:, jt * H + h: jt * H + h + 1],
                scalar2=None,
                op0=OP.max,
            )
            pq = work.tile([128, N], BF16, tag="pq")
            nc.vector.tensor_tensor(pq[:], tq2[:], m_sb[:, jt * N:(jt + 1) * N], op=OP.mult)
            nc.tensor.matmul(
                avp[:, 0:512], lh[:], pq[:, 0:512],
                start=(jt == 0), stop=(jt == NT - 1),
            )
            nc.tensor.matmul(
                avp[:, 512:1024], lh[:], pq[:, 512:1024],
                start=(jt == 0), stop=(jt == NT - 1),
            )
        # epilogue: transpose + normalize
        oT = opool.tile([65, N], FP32, tag="oT")
        nc.scalar.copy(oT[:], avp[:])
        for it in range(NT):
            tps = ps1.tile([128, 65], FP32, tag="ps1")
            nc.tensor.transpose(tps[:], oT[:, it * 128:(it + 1) * 128], eye_f[0:65, 0:65])
            rc = work.tile([128, 1], FP32, tag="rc")
            nc.vector.reciprocal(rc[:], tps[:, 64:65])
            nc.scalar.activation(
                out_sb[:, it * HC + h * C: it * HC + (h + 1) * C],
                tps[:, 0:C],
                AF.Copy,
                scale=rc[:],
            )

    # ---------------- phase D: bias + ELU + store ----------------
    for it in range(NT):
        zb = work.tile([128, HC], FP32, tag="zb")
        nc.gpsimd.tensor_tensor(zb[:], out_sb[:, it * HC:(it + 1) * HC], bias_b[:], op=OP.add)
        nq = work.tile([128, HC], FP32, tag="nq")
        nc.gpsimd.tensor_scalar(out=nq[:], in0=zb[:], scalar1=0.0, scalar2=None, op0=OP.min)
        eq = work.tile([128, HC], FP32, tag="eq")
        nc.scalar.activation(eq[:], nq[:], AF.Exp)
        rq = work.tile([128, HC], FP32, tag="rq")
        nc.gpsimd.tensor_scalar(out=rq[:], in0=zb[:], scalar1=0.0, scalar2=-1.0, op0=OP.max, op1=OP.add)
        nc.vector.tensor_tensor(eq[:], eq[:], rq[:], op=OP.add)
        nc.sync.dma_start(out_d[it * 128:(it + 1) * 128, :], eq[:])


def build_program():
    nc = bacc.Bacc("TRN2", target_bir_lowering=False, debug=False, num_devices=NCORES)
    feat = nc.dram_tensor("feat", [N, D], FP32, kind="ExternalInput").ap()
    adj = nc.dram_tensor("adj", [N, N], I32, kind="ExternalInput").ap()
    w = nc.dram_tensor("w", [D, HC], FP32, kind="ExternalInput").ap()
    wsd = nc.dram_tensor("wsd", [D, 16], FP32, kind="ExternalInput").ap()
    bias_in = nc.dram_tensor("bias", [128, HC], FP32, kind="ExternalInput").ap()
    out_d = nc.dram_tensor("out", [N, HC], FP32, kind="ExternalOutput").ap()
    with tile.TileContext(nc) as tc:
        with ExitStack() as ctx:
            _gat_body(ctx, tc, feat, adj, w, wsd, bias_in, out_d)
    nc.compile()
    return nc


class _Executor:
    """Cached PJRT executor replicating run_bass_via_pjrt's multi-core path,
    so repeated kernel() calls reuse the compiled NEFF."""

    def __init__(self, nc):
        install_neuronx_cc_hook()
        self.nc = nc
        in_names, out_names, out_avals, zero_shapes = [], [], [], []
        partition_name = nc.partition_id_tensor.name if nc.partition_id_tensor else None
        for alloc in nc.m.functions[0].allocations:
            if not isinstance(alloc, mybir.MemoryLocationSet):
                continue
            name = alloc.memorylocations[0].name
            if alloc.kind == "ExternalInput":
                if name != partition_name:
                    in_names.append(name)
            elif alloc.kind == "ExternalOutput":
                shape = tuple(alloc.tensor_shape)
                dtype = mybir.dt.np(alloc.dtype)
                out_names.append(name)
                out_avals.append(jax.core.ShapedArray(shape, dtype))
                zero_shapes.append((shape, dtype))
        self.n_params = len(in_names)
        self.in_names = list(in_names)
        self.out_names = out_names
        self.out_avals = out_avals
        self.zero_shapes = zero_shapes
        all_in_names = in_names + out_names
        if partition_name is not None:
            all_in_names.append(partition_name)
        self.partition_name = partition_name

        out_avals_t = tuple(out_avals)
        all_in_names_t = tuple(all_in_names)
        out_names_t = tuple(out_names)

        def _body(*args):
            operands = list(args)
            if partition_name is not None:
                operands.append(partition_id_tensor())
            outs = _bass_exec_p.bind(
                *operands,
                out_avals=out_avals_t,
                in_names=all_in_names_t,
                out_names=out_names_t,
                lowering_input_output_aliases=(),
                sim_require_finite=True,
                sim_require_nnan=True,
                nc=nc,
            )
            return tuple(outs)

        devices = jax.devices()[:NCORES]
        assert len(devices) == NCORES
        self.mesh = Mesh(np.asarray(devices), ("core",))
        n_outs = len(out_names)
        in_specs = (PartitionSpec("core"),) * (self.n_params + n_outs)
        out_specs = (PartitionSpec("core"),) * n_outs
        self.fn = jax.jit(
            shard_map(_body, mesh=self.mesh, in_specs=in_specs,
                      out_specs=out_specs, check_rep=False),
            keep_unused=True,
        )

    def concat_inputs(self, in_maps):
        return [
            np.concatenate([np.asarray(in_maps[c][nm]) for c in range(NCORES)], axis=0)
            for nm in self.in_names
        ]

    def zeros(self):
        return [
            np.zeros((NCORES * s[0], *s[1:]), dt) for (s, dt) in self.zero_shapes
        ]

    def run(self, concat_in):
        out_arrs = self.fn(*concat_in, *self.zeros())
        return out_arrs

    def device_args(self, concat_in):
        """device_put all operands (inputs + zero output operands) with the
        shard_map sharding so repeated timed calls skip host->device copies."""
        from jax.sharding import NamedSharding
        sh = NamedSharding(self.mesh, PartitionSpec("core"))
        return [jax.device_put(a, sh) for a in (*concat_in, *self.zeros())]

    def run_device(self, dev_args):
        return self.fn(*dev_args)

    def split_outputs(self, out_arrs):
        res = []
        for c in range(NCORES):
            d = {}
            for i, nm in enumerate(self.out_names):
                full = np.asarray(out_arrs[i])
                per = full.reshape(NCORES, *self.out_avals[i].shape)
                d[nm] = per[c]
            res.append(d)
        return res


_EXEC = None


def _get_exec():
    global _EXEC
    if _EXEC is None:
        _EXEC = _Executor(build_program())
    return _EXEC


def _make_in_maps(features_batch, adj_mats_batch, W, att_src, att_dst, bias):
    Wf = np.asarray(W, np.float32)
    asrc = np.asarray(att_src, np.float32)
    adst = np.asarray(att_dst, np.float32)
    Ablk_src = np.zeros((HC, H), np.float32)
    Ablk_dst = np.zeros((HC, H), np.float32)
    for h in range(H):
        Ablk_src[h * C:(h + 1) * C, h] = asrc[h]
        Ablk_dst[h * C:(h + 1) * C, h] = adst[h]
    wsd = np.concatenate([Wf @ Ablk_src, Wf @ Ablk_dst], axis=1)  # [D, 16]
    bias_r = np.ascontiguousarray(
        np.broadcast_to(np.asarray(bias, np.float32).reshape(1, HC), (128, HC))
    )
    in_maps = []
    for c in range(NCORES):
        in_maps.append({
            "feat": np.ascontiguousarray(features_batch[c], dtype=np.float32),
            "adj": np.ascontiguousarray(adj_mats_batch[c], dtype=np.int32),
            "w": Wf,
            "wsd": wsd,
            "bias": bias_r,
        })
    return in_maps


def kernel(features_batch, adj_mats_batch, W, att_src, att_dst, bias):
    ex = _get_exec()
    in_maps = _make_in_maps(features_batch, adj_mats_batch, W, att_src, att_dst, bias)
    concat_in = ex.concat_inputs(in_maps)
    out_arrs = ex.run(concat_in)
    per_core = ex.split_outputs(out_arrs)
    out = np.stack([per_core[c]["out"] for c in range(NCORES)], axis=0)
    return out.astype(np.float32)


# revision 25
# speedup vs baseline: 189650.6502x; 110.9019x over previous
"""Batched GAT (GATConv forward + ELU) Trainium2 Bass kernel.

Problem: B=8 graphs, N=1024 nodes, D=512 features, H=8 heads, C=64 per head.
Sharding: data-parallel, one graph per NeuronCore (8 cores).

Math per graph (reference):
  x = feat @ W                      [N, H*C]
  a_src[n,h] = <x[n,h,:], att_src[h,:]>,  a_dst likewise
  e[i,j,h] = leaky_relu(a_dst[i,h] + a_src[j,h], 0.2)   (edge j->i)
  mask[i,j] = adj[j,i] != 0  or i==j
  alpha = softmax_j(e masked)
  out[i] = elu(concat_h(sum_j alpha[i,j,h] x[j,h,:]) + bias)

Kernel decomposition (per core), working in "transposed" orientation
P_T[j, i] with source nodes j on partitions:
  exp(leaky(s)) = max(exp(s), exp(0.2 s))   with s = a_dst[i] + a_src[j]
  exp(s)      = exp(a_src[j]) * exp(a_dst[i])     (rank-1)
  exp(0.2 s)  = exp(0.2 a_src[j]) * exp(0.2 a_dst[i])
  Factor v1[j]=exp(a_src[j]) into the matmul lhsT and divide the max through
  by exp(0.2 a_dst[i]) (an i-only factor, cancels in the softmax ratio):
    P[j,i] = v1[j] * exp(0.2 a_dst[i]) * P''[j,i]
    P''    = m[j,i] * max(rb[j,i], rho[j])
  where rb broadcasts r[i] = exp(0.8 a_dst[i]) along partitions and
  rho[j] = exp(-0.8 a_src[j]).  max-with-rho is a tensor_scalar op.
  AV matmul: lhsT = [x_h * v1 | v1] (65 cols) -> psum [65, 1024]; row 64 is
  the softmax denominator. Transpose 128-blocks back, multiply by 1/denom,
  then bias + ELU.
"""

import numpy as np
from contextlib import ExitStack

import jax
import numpy as _np
from jax.sharding import Mesh, PartitionSpec
from jax.experimental.shard_map import shard_map

import concourse.bass as bass
import concourse.bacc as bacc
import concourse.tile as tile
from concourse import mybir
from concourse.masks import make_identity
from concourse.bass2jax import (
    _bass_exec_p,
    install_neuronx_cc_hook,
    partition_id_tensor,
)

B, N, D, H, C = 8, 1024, 512, 8, 64
HC = H * C
NCORES = 8
NT = N // 128  # 8 row tiles
KD = D // 128  # 4 contraction tiles

FP32 = mybir.dt.float32
BF16 = mybir.dt.bfloat16
I32 = mybir.dt.int32
AF = mybir.ActivationFunctionType
OP = mybir.AluOpType

# broadcast implementation for exp(0.8 a_dst) rows: "dma1" (seed+log-double
# DMA chain) or "pe" (PE outer product + ACT evict, per-head granular)
BCAST_MODE = "pe"

# engine for the Q = max(rb, rho) tensor_scalar in the hot loop:
# "gpsimd" offloads VE but contends for the shared POOL SBUF port;
# "vector" keeps it on DVE at 4x bf16.
Q_ENGINE = "vector"


def _gat_body(ctx: ExitStack, tc: "tile.TileContext", feat_d, adj_d, w_d, wsd_d, bias_d, oneh_d, eyef_d, eyeb_d, out_d, stage=99):
    nc = tc.nc

    const = ctx.enter_context(tc.tile_pool(name="const", bufs=1))
    stage_p = ctx.enter_context(tc.tile_pool(name="stage", bufs=3))
    big = ctx.enter_context(tc.tile_pool(name="big", bufs=1))
    work = ctx.enter_context(tc.tile_pool(name="work", bufs=3))
    upool = ctx.enter_context(tc.tile_pool(name="u", bufs=1))
    opool = ctx.enter_context(tc.tile_pool(name="o", bufs=2))
    ps1 = ctx.enter_context(tc.tile_pool(name="ps1", bufs=3, space="PSUM"))
    ps2 = ctx.enter_context(tc.tile_pool(name="ps2", bufs=2, space="PSUM"))

    # ---------------- constants / inputs in SBUF ----------------
    eye_f = const.tile([128, 128], FP32)
    nc.sync.dma_start(eye_f[:], eyef_d[:])
    eye_b = const.tile([128, 128], BF16)
    nc.sync.dma_start(eye_b[:], eyeb_d[:])

    w_sb = const.tile([128, KD * HC], FP32)
    for dk in range(KD):
        nc.sync.dma_start(w_sb[:, dk * HC:(dk + 1) * HC], w_d[dk * 128:(dk + 1) * 128, :])
    wsd_sb = const.tile([128, KD * 16], FP32)
    for dk in range(KD):
        nc.sync.dma_start(wsd_sb[:, dk * 16:(dk + 1) * 16], wsd_d[dk * 128:(dk + 1) * 128, :])
    bias_b = const.tile([128, HC], FP32)
    nc.sync.dma_start(bias_b[:], bias_d[:])
    oneh_sb = const.tile([8, H * 128], BF16)
    nc.sync.dma_start(oneh_sb[:], oneh_d[:])

    # ---------------- phase B1: load features per n-tile + transpose ----------------
    # fT[d, n]: KD tiles of [128, N]
    fT_sb = big.tile([128, KD * N], FP32)
    for nt in range(NT if stage >= 1 else 0):
        ftile = stage_p.tile([128, D], FP32, tag="ftile")
        nc.sync.dma_start(ftile[:], feat_d[nt * 128:(nt + 1) * 128, :])
        for dk in range(KD):
            tp = ps1.tile([128, 128], FP32, tag="ps1")
            nc.tensor.transpose(tp[:], ftile[:, dk * 128:(dk + 1) * 128], eye_f[:])
            nc.scalar.copy(fT_sb[:, dk * N + nt * 128: dk * N + (nt + 1) * 128], tp[:])

    # ---------------- phase B2: x projection + attention vectors ----------------
    # xa layout per j-tile: 8 head blocks of 65 cols: [x_h (64) | ones]
    xa_sb = big.tile([128, NT * H * 65], BF16)
    v1_sb = const.tile([128, NT * H], FP32)   # exp(a_src)
    rho_sb = const.tile([128, NT * H], FP32)  # exp(-0.8 a_src)
    avraw_sb = const.tile([128, NT * 16], FP32)  # [a_src | a_dst] per n-tile

    for nt in range(NT if stage >= 2 else 0):
        pp = ps1.tile([128, HC], FP32, tag="ps1")
        for dk in range(KD):
            nc.tensor.matmul(
                pp[:],
                fT_sb[:, dk * N + nt * 128: dk * N + (nt + 1) * 128],
                w_sb[:, dk * HC:(dk + 1) * HC],
                start=(dk == 0),
                stop=(dk == KD - 1),
            )
        xa_slice = xa_sb[:, nt * H * 65:(nt + 1) * H * 65].rearrange("p (h c) -> p h c", h=H)
        nc.scalar.copy(xa_slice[:, :, 0:C], pp[:].rearrange("p (h c) -> p h c", h=H))
        nc.vector.memset(xa_slice[:, :, C:C + 1], 1.0)

        ap_ = ps1.tile([128, 16], FP32, tag="ps1")
        for dk in range(KD):
            nc.tensor.matmul(
                ap_[:],
                fT_sb[:, dk * N + nt * 128: dk * N + (nt + 1) * 128],
                wsd_sb[:, dk * 16:(dk + 1) * 16],
                start=(dk == 0),
                stop=(dk == KD - 1),
            )
        nc.scalar.copy(avraw_sb[:, nt * 16:(nt + 1) * 16], ap_[:])
        nc.scalar.activation(v1_sb[:, nt * H:(nt + 1) * H], ap_[:, 0:H], AF.Exp)
        nc.scalar.activation(rho_sb[:, nt * H:(nt + 1) * H], ap_[:, 0:H], AF.Exp, scale=-0.8)

    # ---------------- phase B3: a_dst rows (transposed) + exp ----------------
    adT_sb = const.tile([8, N], FP32)
    if stage < 2:
        nc.vector.memset(adT_sb[:], 0.0)
    for nt in range(NT if stage >= 2 else 0):
        tq = ps1.tile([8, 128], FP32, tag="ps1")
        nc.tensor.transpose(tq[:], avraw_sb[:, nt * 16 + 8:(nt + 1) * 16], eye_f[:])
        nc.scalar.copy(adT_sb[:, nt * 128:(nt + 1) * 128], tq[:])
    R_sb = const.tile([8, N], BF16)
    nc.scalar.activation(R_sb[:], adT_sb[:], AF.Exp, scale=0.8)  # exp(0.8 a_dst)

    # ---------------- phase B4: mask (adj != 0) | eye, as bf16 ----------------
    m_sb = big.tile([128, NT * N], BF16)
    for jt in range(NT if stage >= 3 else 0):
        atile = stage_p.tile([128, N], I32, tag="atile")
        nc.sync.dma_start(atile[:], adj_d[jt * 128:(jt + 1) * 128, :])
        nc.vector.tensor_scalar(
            out=m_sb[:, jt * N:(jt + 1) * N],
            in0=atile[:],
            scalar1=0,
            scalar2=None,
            op0=OP.not_equal,
        )
        dslice = m_sb[:, jt * N + jt * 128: jt * N + (jt + 1) * 128]
        nc.vector.tensor_tensor(dslice, dslice, eye_b[:], op=OP.max)

    # ---------------- broadcast exp(0.8 a_dst) rows ----------------
    # One [128, H*N] tile: cols h*N..(h+1)*N hold r_h broadcast along partitions.
    r_all = upool.tile([128, H * N], BF16, tag="r_all")
    if stage < 4:
        pass
    elif BCAST_MODE == "dma1":
        # Seed partition 0 with all 8 rows in one DMA (partition-major read of
        # R_sb), then 7 log-doubling SBUF->SBUF DMAs.
        nc.sync.dma_start(r_all[0:1, :], R_sb[:, :])
        k = 1
        while k < 128:
            nc.sync.dma_start(r_all[k:2 * k, :], r_all[0:k, :])
            k *= 2
    else:
        # PE broadcast via one-hot selector: lhsT = oneh[:, h*128:(h+1)*128]
        # ([8, 128], row h all ones), rhs = R_sb [8, 512] -> out[m, n] = r_h[n].
        for h in range(H):
            for half in range(2):
                bp = ps1.tile([128, 512], FP32, tag="ps1")
                nc.tensor.matmul(bp[:], oneh_sb[:, h * 128:(h + 1) * 128],
                                 R_sb[:, half * 512:(half + 1) * 512])
                nc.scalar.copy(r_all[:, h * N + half * 512: h * N + (half + 1) * 512], bp[:])

    # ---------------- phase C: attention + AV ----------------
    out_sb = big.tile([128, NT * HC], FP32)
    if stage < 7:
        nc.vector.memset(out_sb[:], 0.0)
    for h in range(H if stage >= 5 else 0):
        rb = r_all[:, h * N:(h + 1) * N]
        avp = ps2.tile([65, N], FP32, tag="avp")
        for jt in range(NT):
            lh = work.tile([128, 65], BF16, tag="lh")
            nc.vector.tensor_scalar(
                out=lh[:],
                in0=xa_sb[:, jt * H * 65 + h * 65: jt * H * 65 + (h + 1) * 65],
                scalar1=v1_sb[:, jt * H + h: jt * H + h + 1],
                scalar2=None,
                op0=OP.mult,
            )
            tq2 = work.tile([128, N], BF16, tag="tq2")
            q_eng = nc.gpsimd if Q_ENGINE == "gpsimd" else nc.vector
            q_eng.tensor_scalar(
                out=tq2[:],
                in0=rb[:],
                scalar1=rho_sb[:, jt * H + h: jt * H + h + 1],
                scalar2=None,
                op0=OP.max,
            )
            pq = work.tile([128, N], BF16, tag="pq")
            nc.vector.tensor_tensor(pq[:], tq2[:], m_sb[:, jt * N:(jt + 1) * N], op=OP.mult)
            if stage >= 6:
                nc.tensor.matmul(
                    avp[:, 0:512], lh[:], pq[:, 0:512],
                    start=(jt == 0), stop=(jt == NT - 1),
                )
                nc.tensor.matmul(
                    avp[:, 512:1024], lh[:], pq[:, 512:1024],
                    start=(jt == 0), stop=(jt == NT - 1),
                )
        if stage < 7:
            continue
        # epilogue: transpose + normalize
        oT = opool.tile([65, N], FP32, tag="oT")
        nc.scalar.copy(oT[:], avp[:])
        for it in range(NT):
            tps = ps1.tile([128, 65], FP32, tag="ps1")
            nc.tensor.transpose(tps[:], oT[:, it * 128:(it + 1) * 128], eye_f[0:65, 0:65])
            rc = work.tile([128, 1], FP32, tag="rc")
            nc.vector.reciprocal(rc[:], tps[:, 64:65])
            nc.scalar.activation(
                out_sb[:, it * HC + h * C: it * HC + (h + 1) * C],
                tps[:, 0:C],
                AF.Copy,
                scale=rc[:],
            )

    # ---------------- phase D: bias + ELU + store ----------------
    for it in range(NT):
        if stage < 8:
            nc.sync.dma_start(out_d[it * 128:(it + 1) * 128, :], out_sb[:, it * HC:(it + 1) * HC])
            continue
        zb = work.tile([128, HC], FP32, tag="zb")
        nc.vector.tensor_tensor(zb[:], out_sb[:, it * HC:(it + 1) * HC], bias_b[:], op=OP.add)
        nq = work.tile([128, HC], FP32, tag="nq")
        nc.vector.tensor_scalar(out=nq[:], in0=zb[:], scalar1=0.0, scalar2=None, op0=OP.min)
        eq = work.tile([128, HC], FP32, tag="eq")
        nc.scalar.activation(eq[:], nq[:], AF.Exp)
        rq = work.tile([128, HC], FP32, tag="rq")
        nc.vector.tensor_scalar(out=rq[:], in0=zb[:], scalar1=0.0, scalar2=-1.0, op0=OP.max, op1=OP.add)
        nc.vector.tensor_tensor(eq[:], eq[:], rq[:], op=OP.add)
        nc.sync.dma_start(out_d[it * 128:(it + 1) * 128, :], eq[:])


def build_program():
    nc = bacc.Bacc("TRN2", target_bir_lowering=False, debug=False, num_devices=NCORES)
    feat = nc.dram_tensor("feat", [N, D], FP32, kind="ExternalInput").ap()
    adj = nc.dram_tensor("adj", [N, N], I32, kind="ExternalInput").ap()
    w = nc.dram_tensor("w", [D, HC], FP32, kind="ExternalInput").ap()
    wsd = nc.dram_tensor("wsd", [D, 16], FP32, kind="ExternalInput").ap()
    bias_in = nc.dram_tensor("bias", [128, HC], FP32, kind="ExternalInput").ap()
    oneh_in = nc.dram_tensor("oneh", [8, H * 128], BF16, kind="ExternalInput").ap()
    eyef_in = nc.dram_tensor("eyef", [128, 128], FP32, kind="ExternalInput").ap()
    eyeb_in = nc.dram_tensor("eyeb", [128, 128], BF16, kind="ExternalInput").ap()
    out_d = nc.dram_tensor("out", [N, HC], FP32, kind="ExternalOutput").ap()
    with tile.TileContext(nc) as tc:
        with ExitStack() as ctx:
            _gat_body(ctx, tc, feat, adj, w, wsd, bias_in, oneh_in, eyef_in, eyeb_in, out_d)
    nc.compile()
    return nc


class _Executor:
    """Cached PJRT executor replicating run_bass_via_pjrt's multi-core path,
    so repeated kernel() calls reuse the compiled NEFF."""

    def __init__(self, nc):
        install_neuronx_cc_hook()
        self.nc = nc
        in_names, out_names, out_avals, zero_shapes = [], [], [], []
        partition_name = nc.partition_id_tensor.name if nc.partition_id_tensor else None
        for alloc in nc.m.functions[0].allocations:
            if not isinstance(alloc, mybir.MemoryLocationSet):
                continue
            name = alloc.memorylocations[0].name
            if alloc.kind == "ExternalInput":
                if name != partition_name:
                    in_names.append(name)
            elif alloc.kind == "ExternalOutput":
                shape = tuple(alloc.tensor_shape)
                dtype = mybir.dt.np(alloc.dtype)
                out_names.append(name)
                out_avals.append(jax.core.ShapedArray(shape, dtype))
                zero_shapes.append((shape, dtype))
        self.n_params = len(in_names)
        self.in_names = list(in_names)
        self.out_names = out_names
        self.out_avals = out_avals
        self.zero_shapes = zero_shapes
        all_in_names = in_names + out_names
        if partition_name is not None:
            all_in_names.append(partition_name)
        self.partition_name = partition_name

        out_avals_t = tuple(out_avals)
        all_in_names_t = tuple(all_in_names)
        out_names_t = tuple(out_names)

        def _body(*args):
            operands = list(args)
            if partition_name is not None:
                operands.append(partition_id_tensor())
            outs = _bass_exec_p.bind(
                *operands,
                out_avals=out_avals_t,
                in_names=all_in_names_t,
                out_names=out_names_t,
                lowering_input_output_aliases=(),
                sim_require_finite=True,
                sim_require_nnan=True,
                nc=nc,
            )
            return tuple(outs)

        devices = jax.devices()[:NCORES]
        assert len(devices) == NCORES
        self.mesh = Mesh(np.asarray(devices), ("core",))
        n_outs = len(out_names)
        in_specs = (PartitionSpec("core"),) * (self.n_params + n_outs)
        out_specs = (PartitionSpec("core"),) * n_outs
        self.fn = jax.jit(
            shard_map(_body, mesh=self.mesh, in_specs=in_specs,
                      out_specs=out_specs, check_rep=False),
            keep_unused=True,
        )

    def concat_inputs(self, in_maps):
        return [
            np.concatenate([np.asarray(in_maps[c][nm]) for c in range(NCORES)], axis=0)
            for nm in self.in_names
        ]

    def zeros(self):
        return [
            np.zeros((NCORES * s[0], *s[1:]), dt) for (s, dt) in self.zero_shapes
        ]

    def run(self, concat_in):
        out_arrs = self.fn(*concat_in, *self.zeros())
        return out_arrs

    def device_args(self, concat_in):
        """device_put all operands (inputs + zero output operands) with the
        shard_map sharding so repeated timed calls skip host->device copies."""
        from jax.sharding import NamedSharding
        sh = NamedSharding(self.mesh, PartitionSpec("core"))
        return [jax.device_put(a, sh) for a in (*concat_in, *self.zeros())]

    def run_device(self, dev_args):
        return self.fn(*dev_args)

    def split_outputs(self, out_arrs):
        res = []
        for c in range(NCORES):
            d = {}
            for i, nm in enumerate(self.out_names):
                full = np.asarray(out_arrs[i])
                per = full.reshape(NCORES, *self.out_avals[i].shape)
                d[nm] = per[c]
            res.append(d)
        return res


_EXEC = None


def _get_exec():
    global _EXEC
    if _EXEC is None:
        _EXEC = _Executor(build_program())
    return _EXEC


def _make_in_maps(features_batch, adj_mats_batch, W, att_src, att_dst, bias):
    Wf = np.asarray(W, np.float32)
    asrc = np.asarray(att_src, np.float32)
    adst = np.asarray(att_dst, np.float32)
    Ablk_src = np.zeros((HC, H), np.float32)
    Ablk_dst = np.zeros((HC, H), np.float32)
    for h in range(H):
        Ablk_src[h * C:(h + 1) * C, h] = asrc[h]
        Ablk_dst[h * C:(h + 1) * C, h] = adst[h]
    wsd = np.concatenate([Wf @ Ablk_src, Wf @ Ablk_dst], axis=1)  # [D, 16]
    bias_r = np.ascontiguousarray(
        np.broadcast_to(np.asarray(bias, np.float32).reshape(1, HC), (128, HC))
    )
    import ml_dtypes
    oneh = np.zeros((8, H * 128), ml_dtypes.bfloat16)
    for h in range(H):
        oneh[h, h * 128:(h + 1) * 128] = 1.0
    in_maps = []
    for c in range(NCORES):
        in_maps.append({
            "feat": np.ascontiguousarray(features_batch[c], dtype=np.float32),
            "adj": np.ascontiguousarray(adj_mats_batch[c], dtype=np.int32),
            "w": Wf,
            "wsd": wsd,
            "bias": bias_r,
            "oneh": oneh,
            "eyef": np.eye(128, dtype=np.float32),
            "eyeb": np.eye(128).astype(ml_dtypes.bfloat16),
        })
    return in_maps


def kernel(features_batch, adj_mats_batch, W, att_src, att_dst, bias):
    ex = _get_exec()
    in_maps = _make_in_maps(features_batch, adj_mats_batch, W, att_src, att_dst, bias)
    concat_in = ex.concat_inputs(in_maps)
    out_arrs = ex.run(concat_in)
    per_core = ex.split_outputs(out_arrs)
    out = np.stack([per_core[c]["out"] for c in range(NCORES)], axis=0)
    return out.astype(np.float32)
